# revision 4
# baseline (speedup 1.0000x reference)
"""Trainium2 Bass kernel for nn_EnhancedCausalModel — fp16 restructured version.

Computation: MLP (288->128->64->32) on 18 action-variants per (agent,batch,time)
token; softmax-KL between p_with and averaged p_without finishes on the host in
float64 from fp32 logits.

Sharding: agents (N=16) over 8 cores, 2 agents/core -> TOK=2048 tokens/core,
processed in 4 tiles of 512 tokens.

Design (per 512-token tile, all matmul operands fp16, PSUM fp32):
  ps_h   = W1a.T @ obs                       (K=256 as 2 accumulating MMs)
  hb     = ps_h + b1                         (DVE drain, fp16)
  17 action variants in 9 pairs; per variant:
    eye-path : PSUM = W1b.T a_v (row-tiled K=32) + I @ hb  -> Act drain relu
    TT-path  : PSUM = W1b.T a_v ; DVE adds hb on drain; Pool relu
  L2: two variants col-tiled per PSUM bank (tile_position (0,0)/(0,64)),
      drained with fused +b2/relu on Act or DVE.
  L3: pairs stacked on partitions, W3 duplicated on K: one MM per pair
      accumulates into a single logits bank (pw / woA / woB slots).
Host: wo = woA + woB; float64 softmax-KL.
"""

import contextlib

import numpy as np

import concourse.bass as bass
import concourse.mybir as mybir
import concourse.tile as tile
from concourse import bacc
from concourse.bass_utils import run_bass_kernel_spmd

F32 = mybir.dt.float32
F16 = mybir.dt.float16
AF = mybir.ActivationFunctionType
ALU = mybir.AluOpType

B, T = 16, 64
N_AG, D_OBS, D_ACT = 16, 256, 32
S_CF = 16
H1, H2 = 128, 64
N_CORES = 8
AG_PER = N_AG // N_CORES          # 2 agents per core
TOK = AG_PER * B * T              # 2048 tokens per core
TILE = 512
NT = TOK // TILE                  # 4 tiles
NV = 17                           # with + 16 cf action variants
NBLK = 5                          # action blocks per tile (4 variants each)
INW = (2 + NBLK) * TILE           # input cols per tile (3584)
INV_S1 = 1.0 / (S_CF + 1)

# fp16 const block column offsets
C_W1A0, C_W1A1, C_W1B4, C_EYE, C_W2, C_W3F, C_W3P = 0, 128, 256, 384, 512, 576, 672
C_TOT = 704

# All 9 variant pairs use the eye-matmul path: the h_obs (+b1) add happens in
# PSUM via an identity matmul, so every L1 drain is a single fused bias+relu
# op, alternating between DVE and Act per pair.


def build_nc(reps=None, unroll=1):
    nc = bacc.Bacc("TRN2", target_bir_lowering=False, debug=False,
                   num_devices=N_CORES)

    cblk = nc.dram_tensor("cblk", [128, C_TOT], F16, kind="ExternalInput").ap()
    cblk2 = nc.dram_tensor("cblk2", [128, 2], F32, kind="ExternalInput").ap()
    inblk = nc.dram_tensor("inblk", [128, NT * INW], F16,
                           kind="ExternalInput").ap()
    logits = nc.dram_tensor("logits", [96, NT, TILE], F32,
                            kind="ExternalOutput").ap()

    with tile.TileContext(nc) as tc:
        with (
            tc.tile_pool(name="const", bufs=1) as cpool,
            tc.tile_pool(name="inp", bufs=2) as ipool,
            tc.tile_pool(name="hb", bufs=2) as hbpool,
            tc.tile_pool(name="h1", bufs=3) as h1pool,
            tc.tile_pool(name="h2", bufs=3) as h2pool,
            tc.tile_pool(name="hz", bufs=2) as hzpool,
            tc.tile_pool(name="lg", bufs=1) as lgpool,
            tc.tile_pool(name="ph", bufs=1, space="PSUM") as php,
            tc.tile_pool(name="p1", bufs=2, space="PSUM") as p1p,
            tc.tile_pool(name="p2", bufs=2, space="PSUM") as p2p,
            tc.tile_pool(name="plg", bufs=1, space="PSUM") as plgp,
        ):
            cb = cpool.tile([128, C_TOT], F16)
            nc.sync.dma_start(out=cb[:], in_=cblk[:])
            cb2 = cpool.tile([128, 2], F32)
            nc.sync.dma_start(out=cb2[:], in_=cblk2[:])
            w1a0 = cb[:, C_W1A0:C_W1A0 + 128]
            w1a1 = cb[:, C_W1A1:C_W1A1 + 128]
            w1b4 = cb[:, C_W1B4:C_W1B4 + 128]
            eye = cb[:, C_EYE:C_EYE + 128]
            w2 = cb[:, C_W2:C_W2 + 64]
            w3f = cb[:, C_W3F:C_W3F + 96]
            w3p = cb[:, C_W3P:C_W3P + 32]
            b1 = cb2[:, 0:1]
            b2r = cb2[:, 1:2]

            lg = lgpool.tile([96, NT, TILE], F32)
            mm = nc.tensor.matmul

            def emit_tile(i):
                inb = ipool.tile([128, INW], F16, tag="inb")
                nc.sync.dma_start(out=inb[:, 0:1792],
                                  in_=inblk[:, i * INW:i * INW + 1792])
                nc.sync.dma_start(out=inb[:, 1792:INW],
                                  in_=inblk[:, i * INW + 1792:(i + 1) * INW])
                obs0 = inb[:, 0:TILE]
                obs1 = inb[:, TILE:2 * TILE]

                # h_obs (+b1 folded on drain) -> hb fp16
                ps_h = php.tile([128, TILE], F32, tag="ph")
                mm(ps_h[:], w1a0, obs0, start=True, stop=False)
                mm(ps_h[:], w1a1, obs1, start=False, stop=True)
                hb = hbpool.tile([128, TILE], F16, tag="hb")
                nc.vector.tensor_scalar(hb[:], ps_h[:], b1, None, op0=ALU.add)

                # zero-action variant h1
                h1z = hzpool.tile([128, TILE], F16, tag="hz")
                nc.gpsimd.tensor_scalar(h1z[:], hb[:], 0.0, None, op0=ALU.max)

                plg = plgp.tile([128, TILE], F32, tag="plg")

                # Software-pipelined pair stages: while pair p's L1 matmuls
                # run, pair p-1's L2 and pair p-2's L3 interleave on PE so it
                # never stalls waiting for a drain of the same pair.
                def stage_l1(p):
                    pair_n = 1 if p == 8 else 2
                    eye_path = (p % 2 == 0)  # incl p8; odd pairs use DVE+Pool
                    pv = p1p.tile([128, 2 * TILE], F32, tag="p1")
                    # both action matmuls first: different row groups and
                    # different PSUM banks, so they run concurrently; the
                    # full-array eye matmuls follow and accumulate.
                    for s in range(pair_n):
                        v = 2 * p + s
                        r = 32 * (v % 4)
                        blk = v // 4
                        acols = slice(2 * TILE + blk * TILE,
                                      2 * TILE + (blk + 1) * TILE)
                        half = pv[:, s * TILE:(s + 1) * TILE]
                        mm(half, w1b4[r:r + 32, :], inb[r:r + 32, acols],
                           tile_position=(r, 0), start=True, stop=not eye_path)
                    if eye_path:
                        for s in range(pair_n):
                            half = pv[:, s * TILE:(s + 1) * TILE]
                            mm(half, eye, hb[:], start=False, stop=True)
                    h1 = h1pool.tile([128, 2 * TILE], F16, tag="h1")
                    sl = slice(0, TILE if p == 8 else 2 * TILE)
                    if eye_path:
                        nc.scalar.activation(h1[:, sl], pv[:, sl], AF.Relu,
                                             bias=0.0)
                    else:
                        nc.vector.tensor_tensor(h1[:, 0:TILE], pv[:, 0:TILE],
                                                hb[:], op=ALU.add)
                        nc.vector.tensor_tensor(h1[:, TILE:], pv[:, TILE:],
                                                hb[:], op=ALU.add)
                        nc.gpsimd.tensor_scalar(h1[:, sl], h1[:, sl], 0.0,
                                                None, op0=ALU.max)
                    return h1

                def stage_l2(p, h1):
                    q = p2p.tile([128, TILE], F32, tag="p2")
                    mm(q[0:64, :], w2, h1[:, 0:TILE], tile_position=(0, 0))
                    rhsB = h1z[:] if p == 8 else h1[:, TILE:2 * TILE]
                    mm(q[64:128, :], w2, rhsB, tile_position=(0, 64))
                    h2 = h2pool.tile([128, TILE], F16, tag="h2")
                    if p % 2 == 0:
                        nc.scalar.activation(h2[:], q[:], AF.Relu, bias=b2r)
                    else:
                        nc.vector.tensor_scalar(h2[:], q[:], b2r, 0.0,
                                                op0=ALU.add, op1=ALU.max)
                    return h2

                def stage_l3(p, h2):
                    # The first matmul spans all 96 output partitions (zero
                    # block for woB) so later accumulates hit initialized,
                    # has_written-set elements.
                    if p == 0:
                        mm(plg[0:96, :], w3f, h2[:], start=True, stop=False,
                           skip_group_check=True)
                    else:
                        pos = 32 if p % 2 == 1 else 64
                        mm(plg[pos:pos + 32, :], w3p, h2[:],
                           tile_position=(0, pos), start=False, stop=(p == 8),
                           skip_group_check=True)

                h1s, h2s = {}, {}
                for pp in range(11):
                    if pp < 9:
                        h1s[pp] = stage_l1(pp)
                    if 1 <= pp:
                        pprev = pp - 1
                        if pprev < 9:
                            h2s[pprev] = stage_l2(pprev, h1s.pop(pprev))
                    if 2 <= pp:
                        pold = pp - 2
                        if pold < 9:
                            stage_l3(pold, h2s.pop(pold))

                nc.scalar.activation(lg[:, i, :], plg[0:96, :], AF.Copy)
                nc.sync.dma_start(out=logits[:, i, :], in_=lg[:, i, :])

            rep_ctx = (tc.For_i(0, reps, 1,
                                hint_engines=(mybir.EngineType.PE,))
                       if reps else contextlib.nullcontext())
            with rep_ctx:
                for _u in range(unroll):
                    for i in range(NT):
                        emit_tile(i)

    nc.compile()
    return nc


def prep_shared(W1, b1, W2, b2, W3, b3):
    cblk = np.zeros((128, C_TOT), np.float32)
    cblk[:, C_W1A0:C_W1A0 + 128] = W1[0:128]
    cblk[:, C_W1A1:C_W1A1 + 128] = W1[128:256]
    w1b = W1[D_OBS:]
    cblk[:, C_W1B4:C_W1B4 + 128] = np.vstack([w1b] * 4)
    cblk[:, C_EYE:C_EYE + 128] = np.eye(128, dtype=np.float32)
    cblk[:, C_W2:C_W2 + 64] = W2
    cblk[0:64, C_W3F:C_W3F + 32] = W3
    cblk[64:128, C_W3F + 32:C_W3F + 64] = W3
    # cols C_W3F+64 .. C_W3F+96 stay zero: initializes the woB slot
    cblk[0:64, C_W3P:C_W3P + 32] = W3
    cblk[64:128, C_W3P:C_W3P + 32] = W3
    cblk2 = np.zeros((128, 2), np.float32)
    cblk2[:, 0] = b1
    cblk2[0:64, 1] = b2
    cblk2[64:128, 1] = b2
    return dict(cblk=cblk.astype(np.float16), cblk2=cblk2)


def prep_core(obs, actions, cf_actions, c):
    n0 = AG_PER * c
    obs_t = np.transpose(obs[:, :, n0:n0 + AG_PER, :],
                         (3, 2, 0, 1)).reshape(D_OBS, TOK)
    act_w = np.transpose(actions[:, :, n0:n0 + AG_PER, :],
                         (3, 2, 0, 1)).reshape(D_ACT, TOK)
    cf_tok = np.transpose(cf_actions[n0:n0 + AG_PER],
                          (4, 1, 0, 2, 3)).reshape(D_ACT, S_CF, TOK)
    vlist = [act_w] + [cf_tok[:, s] for s in range(S_CF)]
    inblk = np.zeros((128, NT * INW), np.float32)
    for i in range(NT):
        tokc = slice(i * TILE, (i + 1) * TILE)
        base = i * INW
        inblk[0:128, base:base + TILE] = obs_t[0:128, tokc]
        inblk[0:128, base + TILE:base + 2 * TILE] = obs_t[128:256, tokc]
        for v in range(NV):
            blk, r = v // 4, 32 * (v % 4)
            cols = slice(base + 2 * TILE + blk * TILE,
                         base + 2 * TILE + (blk + 1) * TILE)
            inblk[r:r + 32, cols] = vlist[v][:, tokc]
    return dict(inblk=inblk.astype(np.float16))


def logits_to_influence(lg, b3):
    """lg: [96, NT, 512] fp32 device output -> influence [TOK] (float64)."""
    lg = lg.astype(np.float64).reshape(96, TOK)
    pw = lg[0:32]
    wo = lg[32:64] + lg[64:96]
    b3c = b3.astype(np.float64)[:, None]
    w = pw + b3c
    z = wo * INV_S1 + b3c
    e_z = np.exp(z)
    e_w = np.exp(w)
    ez = e_z.sum(axis=0)
    ew = e_w.sum(axis=0)
    s1 = (e_z * (z - w)).sum(axis=0)
    return ((s1 / ez - np.log(ez) + np.log(ew)) / float(D_ACT)).astype(np.float32)


_NC_CACHE = {}


def run_device(inputs, trace=False):
    if "nc" not in _NC_CACHE:
        _NC_CACHE["nc"] = build_nc()
    nc = _NC_CACHE["nc"]
    shared = prep_shared(np.asarray(inputs["W1"]), np.asarray(inputs["b1"]),
                         np.asarray(inputs["W2"]), np.asarray(inputs["b2"]),
                         np.asarray(inputs["W3"]), np.asarray(inputs["b3"]))
    maps = []
    for c in range(N_CORES):
        m = dict(shared)
        m.update(prep_core(np.asarray(inputs["obs"]),
                           np.asarray(inputs["actions"]),
                           np.asarray(inputs["cf_actions"]), c))
        maps.append(m)
    res = run_bass_kernel_spmd(nc, maps, list(range(N_CORES)), trace=trace)
    return res


def kernel(**inputs):
    res = run_device(inputs, trace=False)
    b3 = np.asarray(inputs["b3"])
    out = np.empty((B, T, N_AG), np.float32)
    for c in range(N_CORES):
        infl = logits_to_influence(res.results[c]["logits"], b3)
        r = infl.reshape(AG_PER, B, T)
        for a in range(AG_PER):
            out[:, :, AG_PER * c + a] = r[a]
    return out


# revision 6
# speedup vs baseline: 1.0501x; 1.0501x over previous
"""Trainium2 Bass kernel for nn_EnhancedCausalModel — fp16 restructured version.

Computation: MLP (288->128->64->32) on 18 action-variants per (agent,batch,time)
token; softmax-KL between p_with and averaged p_without finishes on the host in
float64 from fp32 logits.

Sharding: agents (N=16) over 8 cores, 2 agents/core -> TOK=2048 tokens/core,
processed in 4 tiles of 512 tokens.

Design (per 512-token tile, all matmul operands fp16, PSUM fp32):
  ps_h   = W1a.T @ obs                       (K=256 as 2 accumulating MMs)
  hb     = ps_h + b1                         (DVE drain, fp16)
  17 action variants in 9 pairs; per variant:
    eye-path : PSUM = W1b.T a_v (row-tiled K=32) + I @ hb  -> Act drain relu
    TT-path  : PSUM = W1b.T a_v ; DVE adds hb on drain; Pool relu
  L2: two variants col-tiled per PSUM bank (tile_position (0,0)/(0,64)),
      drained with fused +b2/relu on Act or DVE.
  L3: pairs stacked on partitions, W3 duplicated on K: one MM per pair
      accumulates into a single logits bank (pw / woA / woB slots).
Host: wo = woA + woB; float64 softmax-KL.
"""

import contextlib

import numpy as np

import concourse.bass as bass
import concourse.mybir as mybir
import concourse.tile as tile
from concourse import bacc
from concourse.bass_utils import run_bass_kernel_spmd

F32 = mybir.dt.float32
F16 = mybir.dt.float16
AF = mybir.ActivationFunctionType
ALU = mybir.AluOpType

B, T = 16, 64
N_AG, D_OBS, D_ACT = 16, 256, 32
S_CF = 16
H1, H2 = 128, 64
N_CORES = 8
AG_PER = N_AG // N_CORES          # 2 agents per core
TOK = AG_PER * B * T              # 2048 tokens per core
TILE = 512
NT = TOK // TILE                  # 4 tiles
NV = 17                           # with + 16 cf action variants
NBLK = 5                          # action blocks per tile (4 variants each)
INW = (2 + NBLK) * TILE           # input cols per tile (3584)
INV_S1 = 1.0 / (S_CF + 1)

# fp16 const block column offsets
C_W1A0, C_W1A1, C_W1B4, C_EYE, C_W2, C_W3F, C_W3P = 0, 128, 256, 384, 512, 576, 672
C_TOT = 704

# Even variant pairs (incl. the last, single-variant one) use the eye-matmul
# path: the h_obs(+b1) add happens in PSUM via an identity matmul so the L1
# drain is one fused relu on Act. Odd pairs skip the eye matmuls; DVE adds hb
# while draining and Pool applies the relu. This balances PE against the two
# PSUM-drain engines (DVE/Act), with Pool absorbing SBUF-side relu work.


def build_nc(reps=None, unroll=1):
    nc = bacc.Bacc("TRN2", target_bir_lowering=False, debug=False,
                   num_devices=N_CORES)

    cblk = nc.dram_tensor("cblk", [128, C_TOT], F16, kind="ExternalInput").ap()
    cblk2 = nc.dram_tensor("cblk2", [128, 2], F32, kind="ExternalInput").ap()
    inblk = nc.dram_tensor("inblk", [128, NT * INW], F16,
                           kind="ExternalInput").ap()
    logits = nc.dram_tensor("logits", [96, NT, TILE], F32,
                            kind="ExternalOutput").ap()

    with tile.TileContext(nc) as tc:
        with (
            tc.tile_pool(name="const", bufs=1) as cpool,
            tc.tile_pool(name="inp", bufs=2) as ipool,
            tc.tile_pool(name="hb", bufs=2) as hbpool,
            tc.tile_pool(name="h1", bufs=3) as h1pool,
            tc.tile_pool(name="h2", bufs=3) as h2pool,
            tc.tile_pool(name="hz", bufs=2) as hzpool,
            tc.tile_pool(name="lg", bufs=1) as lgpool,
            tc.tile_pool(name="ph", bufs=1, space="PSUM") as php,
            tc.tile_pool(name="p1", bufs=2, space="PSUM") as p1p,
            tc.tile_pool(name="p2", bufs=2, space="PSUM") as p2p,
            tc.tile_pool(name="plg", bufs=1, space="PSUM") as plgp,
        ):
            cb = cpool.tile([128, C_TOT], F16)
            nc.sync.dma_start(out=cb[:, 0:256], in_=cblk[:, 0:256])
            cb2 = cpool.tile([128, 2], F32)
            nc.sync.dma_start(out=cb2[:], in_=cblk2[:])
            nc.gpsimd.dma_start(out=cb[:, 256:C_TOT], in_=cblk[:, 256:C_TOT])
            w1a0 = cb[:, C_W1A0:C_W1A0 + 128]
            w1a1 = cb[:, C_W1A1:C_W1A1 + 128]
            w1b4 = cb[:, C_W1B4:C_W1B4 + 128]
            eye = cb[:, C_EYE:C_EYE + 128]
            w2 = cb[:, C_W2:C_W2 + 64]
            w3f = cb[:, C_W3F:C_W3F + 96]
            w3p = cb[:, C_W3P:C_W3P + 32]
            b1 = cb2[:, 0:1]
            b2r = cb2[:, 1:2]

            lg = lgpool.tile([96, NT, TILE], F32)
            mm = nc.tensor.matmul

            def emit_tile(i):
                inb = ipool.tile([128, INW], F16, tag="inb")
                # obs first (small) so h_obs starts early; the tail block
                # rides the Pool engine's DMA queue to run in parallel with
                # the SP queue.
                nc.sync.dma_start(out=inb[:, 0:1024],
                                  in_=inblk[:, i * INW:i * INW + 1024])
                nc.sync.dma_start(out=inb[:, 1024:2304],
                                  in_=inblk[:, i * INW + 1024:i * INW + 2304])
                nc.gpsimd.dma_start(out=inb[:, 2304:INW],
                                    in_=inblk[:, i * INW + 2304:(i + 1) * INW])
                obs0 = inb[:, 0:TILE]
                obs1 = inb[:, TILE:2 * TILE]

                # h_obs (+b1 folded on drain) -> hb fp16
                ps_h = php.tile([128, TILE], F32, tag="ph")
                mm(ps_h[:], w1a0, obs0, start=True, stop=False)
                mm(ps_h[:], w1a1, obs1, start=False, stop=True)
                hb = hbpool.tile([128, TILE], F16, tag="hb")
                nc.vector.tensor_scalar(hb[:], ps_h[:], b1, None, op0=ALU.add)

                # zero-action variant h1
                h1z = hzpool.tile([128, TILE], F16, tag="hz")
                nc.gpsimd.tensor_scalar(h1z[:], hb[:], 0.0, None, op0=ALU.max)

                plg = plgp.tile([128, TILE], F32, tag="plg")

                # Software-pipelined pair stages: while pair p's L1 matmuls
                # run, pair p-1's L2 and pair p-2's L3 interleave on PE so it
                # never stalls waiting for a drain of the same pair.
                def stage_l1(p):
                    pair_n = 1 if p == 8 else 2
                    eye_path = (p % 2 == 0)  # incl p8; odd pairs use DVE+Pool
                    pv = p1p.tile([128, 2 * TILE], F32, tag="p1")
                    # both action matmuls first: different row groups and
                    # different PSUM banks, so they run concurrently; the
                    # full-array eye matmuls follow and accumulate.
                    for s in range(pair_n):
                        v = 2 * p + s
                        r = 32 * (v % 4)
                        blk = v // 4
                        acols = slice(2 * TILE + blk * TILE,
                                      2 * TILE + (blk + 1) * TILE)
                        half = pv[:, s * TILE:(s + 1) * TILE]
                        mm(half, w1b4[r:r + 32, :], inb[r:r + 32, acols],
                           tile_position=(r, 0), start=True, stop=not eye_path)
                    if eye_path:
                        for s in range(pair_n):
                            half = pv[:, s * TILE:(s + 1) * TILE]
                            mm(half, eye, hb[:], start=False, stop=True)
                    h1 = h1pool.tile([128, 2 * TILE], F16, tag="h1")
                    sl = slice(0, TILE if p == 8 else 2 * TILE)
                    if eye_path:
                        nc.scalar.activation(h1[:, sl], pv[:, sl], AF.Relu,
                                             bias=0.0)
                    else:
                        nc.vector.tensor_tensor(h1[:, 0:TILE], pv[:, 0:TILE],
                                                hb[:], op=ALU.add)
                        nc.vector.tensor_tensor(h1[:, TILE:], pv[:, TILE:],
                                                hb[:], op=ALU.add)
                        nc.gpsimd.tensor_scalar(h1[:, sl], h1[:, sl], 0.0,
                                                None, op0=ALU.max)
                    return h1

                def stage_l2(p, h1):
                    q = p2p.tile([128, TILE], F32, tag="p2")
                    mm(q[0:64, :], w2, h1[:, 0:TILE], tile_position=(0, 0))
                    rhsB = h1z[:] if p == 8 else h1[:, TILE:2 * TILE]
                    mm(q[64:128, :], w2, rhsB, tile_position=(0, 64))
                    h2 = h2pool.tile([128, TILE], F16, tag="h2")
                    if p % 2 == 0:
                        nc.scalar.activation(h2[:], q[:], AF.Relu, bias=b2r)
                    else:
                        nc.vector.tensor_scalar(h2[:], q[:], b2r, 0.0,
                                                op0=ALU.add, op1=ALU.max)
                    return h2

                def stage_l3(p, h2):
                    # The first matmul spans all 96 output partitions (zero
                    # block for woB) so later accumulates hit initialized,
                    # has_written-set elements.
                    if p == 0:
                        mm(plg[0:96, :], w3f, h2[:], start=True, stop=False,
                           skip_group_check=True)
                    else:
                        pos = 32 if p % 2 == 1 else 64
                        mm(plg[pos:pos + 32, :], w3p, h2[:],
                           tile_position=(0, pos), start=False, stop=(p == 8),
                           skip_group_check=True)

                h1s, h2s = {}, {}
                for pp in range(11):
                    if pp < 9:
                        h1s[pp] = stage_l1(pp)
                    if 1 <= pp:
                        pprev = pp - 1
                        if pprev < 9:
                            h2s[pprev] = stage_l2(pprev, h1s.pop(pprev))
                    if 2 <= pp:
                        pold = pp - 2
                        if pold < 9:
                            stage_l3(pold, h2s.pop(pold))

                nc.scalar.activation(lg[:, i, :], plg[0:96, :], AF.Copy)
                nc.gpsimd.dma_start(out=logits[:, i, :], in_=lg[:, i, :])

            rep_ctx = (tc.For_i(0, reps, 1,
                                hint_engines=(mybir.EngineType.PE,))
                       if reps else contextlib.nullcontext())
            with rep_ctx:
                for _u in range(unroll):
                    for i in range(NT):
                        emit_tile(i)

    nc.compile()
    return nc


def prep_shared(W1, b1, W2, b2, W3, b3):
    cblk = np.zeros((128, C_TOT), np.float32)
    cblk[:, C_W1A0:C_W1A0 + 128] = W1[0:128]
    cblk[:, C_W1A1:C_W1A1 + 128] = W1[128:256]
    w1b = W1[D_OBS:]
    cblk[:, C_W1B4:C_W1B4 + 128] = np.vstack([w1b] * 4)
    cblk[:, C_EYE:C_EYE + 128] = np.eye(128, dtype=np.float32)
    cblk[:, C_W2:C_W2 + 64] = W2
    cblk[0:64, C_W3F:C_W3F + 32] = W3
    cblk[64:128, C_W3F + 32:C_W3F + 64] = W3
    # cols C_W3F+64 .. C_W3F+96 stay zero: initializes the woB slot
    cblk[0:64, C_W3P:C_W3P + 32] = W3
    cblk[64:128, C_W3P:C_W3P + 32] = W3
    cblk2 = np.zeros((128, 2), np.float32)
    cblk2[:, 0] = b1
    cblk2[0:64, 1] = b2
    cblk2[64:128, 1] = b2
    return dict(cblk=cblk.astype(np.float16), cblk2=cblk2)


def prep_core(obs, actions, cf_actions, c):
    n0 = AG_PER * c
    obs_t = np.transpose(obs[:, :, n0:n0 + AG_PER, :],
                         (3, 2, 0, 1)).reshape(D_OBS, TOK)
    act_w = np.transpose(actions[:, :, n0:n0 + AG_PER, :],
                         (3, 2, 0, 1)).reshape(D_ACT, TOK)
    cf_tok = np.transpose(cf_actions[n0:n0 + AG_PER],
                          (4, 1, 0, 2, 3)).reshape(D_ACT, S_CF, TOK)
    vlist = [act_w] + [cf_tok[:, s] for s in range(S_CF)]
    inblk = np.zeros((128, NT * INW), np.float32)
    for i in range(NT):
        tokc = slice(i * TILE, (i + 1) * TILE)
        base = i * INW
        inblk[0:128, base:base + TILE] = obs_t[0:128, tokc]
        inblk[0:128, base + TILE:base + 2 * TILE] = obs_t[128:256, tokc]
        for v in range(NV):
            blk, r = v // 4, 32 * (v % 4)
            cols = slice(base + 2 * TILE + blk * TILE,
                         base + 2 * TILE + (blk + 1) * TILE)
            inblk[r:r + 32, cols] = vlist[v][:, tokc]
    return dict(inblk=inblk.astype(np.float16))


def logits_to_influence(lg, b3):
    """lg: [96, NT, 512] fp32 device output -> influence [TOK] (float64)."""
    lg = lg.astype(np.float64).reshape(96, TOK)
    pw = lg[0:32]
    wo = lg[32:64] + lg[64:96]
    b3c = b3.astype(np.float64)[:, None]
    w = pw + b3c
    z = wo * INV_S1 + b3c
    e_z = np.exp(z)
    e_w = np.exp(w)
    ez = e_z.sum(axis=0)
    ew = e_w.sum(axis=0)
    s1 = (e_z * (z - w)).sum(axis=0)
    return ((s1 / ez - np.log(ez) + np.log(ew)) / float(D_ACT)).astype(np.float32)


_NC_CACHE = {}


def run_device(inputs, trace=False):
    if "nc" not in _NC_CACHE:
        _NC_CACHE["nc"] = build_nc()
    nc = _NC_CACHE["nc"]
    shared = prep_shared(np.asarray(inputs["W1"]), np.asarray(inputs["b1"]),
                         np.asarray(inputs["W2"]), np.asarray(inputs["b2"]),
                         np.asarray(inputs["W3"]), np.asarray(inputs["b3"]))
    maps = []
    for c in range(N_CORES):
        m = dict(shared)
        m.update(prep_core(np.asarray(inputs["obs"]),
                           np.asarray(inputs["actions"]),
                           np.asarray(inputs["cf_actions"]), c))
        maps.append(m)
    res = run_bass_kernel_spmd(nc, maps, list(range(N_CORES)), trace=trace)
    return res


def kernel(**inputs):
    res = run_device(inputs, trace=False)
    b3 = np.asarray(inputs["b3"])
    out = np.empty((B, T, N_AG), np.float32)
    for c in range(N_CORES):
        infl = logits_to_influence(res.results[c]["logits"], b3)
        r = infl.reshape(AG_PER, B, T)
        for a in range(AG_PER):
            out[:, :, AG_PER * c + a] = r[a]
    return out
